# revision 1
# baseline (speedup 1.0000x reference)
"""Multi-head causal self-attention (B=2, T=4096, D=768, H=12) on 8 trn2 cores.

Sharding: core c -> batch b = c//4, heads 3*(c%4) .. 3*(c%4)+2.
qkv_proj column-parallel (each core computes Q/K/V only for its heads),
out_proj row-parallel (each core emits a partial y^T; host sums the 4
partials per batch).

Device dataflow (all fp32):
  x^T tiles via PE transposes -> Q^T/K^T via transposed projection
  (W^T stationary, x^T streaming) -> S^T = K Q^T in [k,q] layout, two
  heads row-paired on opposite PE halves -> exp on ScalarE (no max
  subtraction; scores ~ N(0,1)) -> causal band masks on DVE ->
  out^T = V^T P^T col-paired (even k-tiles -> psum partitions 0:64,
  odd -> 64:128) with a parallel 4-way col-tiled ones-matmul computing
  softmax denominators -> normalize via batched reciprocal + gpsimd
  partition broadcast -> y^T = Wo^T.T out^T with heads 0/1 row-paired.
"""

import sys

sys.path.insert(0, "/opt/trn_rl_repo")

import numpy as np
from contextlib import ExitStack

import concourse.bass as bass
import concourse.bacc as bacc
import concourse.tile as tile
import concourse.mybir as mybir
from concourse.masks import make_identity
from concourse.bass_utils import run_bass_kernel_spmd

F32 = mybir.dt.float32
AF = mybir.ActivationFunctionType

B = 2
T = 4096
D = 768
H = 12
DK = 64
NCORES = 8
HL = 3  # heads per core
ND = D // 128  # 6 d-tiles
NKT = T // 128  # 32 k-tiles
NQB = T // 512  # 8 q-blocks
NTSB = T // 512  # 8 t-superblocks (4 t-tiles each)

_CACHE = {}
USE_PB = True  # gpsimd partition_broadcast for the reciprocal broadcast


def _emit(tc):
    nc = tc.nc
    x_d = nc.dram_tensor("x", [T, D], F32, kind="ExternalInput").ap()
    wqk_d = nc.dram_tensor("wqkT", [D, 384], F32, kind="ExternalInput").ap()
    wv_d = nc.dram_tensor("wvT", [D, HL * DK], F32, kind="ExternalInput").ap()
    wo_d = nc.dram_tensor("woT", [HL, DK, D], F32, kind="ExternalInput").ap()
    y_d = nc.dram_tensor("yT", [D, T], F32, kind="ExternalOutput").ap()

    ctx = ExitStack()
    const = ctx.enter_context(tc.tile_pool(name="const", bufs=1))
    persist = ctx.enter_context(tc.tile_pool(name="persist", bufs=1))
    xpool = ctx.enter_context(tc.tile_pool(name="xp", bufs=2))
    xtpool = ctx.enter_context(tc.tile_pool(name="xt", bufs=1))
    ptpool = ctx.enter_context(tc.tile_pool(name="pt", bufs=5))
    spool = ctx.enter_context(tc.tile_pool(name="sp", bufs=2))
    otpool = ctx.enter_context(tc.tile_pool(name="ot", bufs=1))
    ypool = ctx.enter_context(tc.tile_pool(name="yp", bufs=2))
    # PSUM: pa = streaming (S tiles, transposes, qkv, V); pb = AV
    # accumulators (one [128,512] bank per head, even/odd halves);
    # pc = sums accumulators + reduce + yT.
    psA = ctx.enter_context(tc.tile_pool(name="psA", bufs=2, space="PSUM"))
    psB = ctx.enter_context(tc.tile_pool(name="psB", bufs=3, space="PSUM"))
    psC = ctx.enter_context(tc.tile_pool(name="psC", bufs=1, space="PSUM"))

    # ---- constants ----
    ident = const.tile([128, 128], F32)
    make_identity(nc, ident)
    # causal band masks for the 4 diagonal-band k-tiles of each q-block:
    # bandmask[bp][k, q] = 0 for q < 128*bp + k, else 1
    bandmask = []
    for bp in range(4):
        m = const.tile([128, 512], F32, name=f"bandmask{bp}")
        nc.gpsimd.memset(m, 1.0)
        nc.gpsimd.affine_select(
            out=m, in_=m, compare_op=mybir.AluOpType.is_ge, fill=0.0,
            base=-128 * bp, pattern=[[1, 512]], channel_multiplier=-1,
        )
        bandmask.append(m)
    ones1 = const.tile([128, 1], F32)
    nc.vector.memset(ones1, 1.0)
    ones64 = const.tile([1, DK], F32)
    nc.vector.memset(ones64, 1.0)
    ones4 = const.tile([128, 1], F32)
    nc.vector.memset(ones4, 0.0)
    for r in (0, 32, 64, 96):
        nc.vector.memset(ones4[r : r + 1, :], 1.0)

    wqk_sb = const.tile([128, ND, 384], F32)
    nc.sync.dma_start(out=wqk_sb, in_=wqk_d.rearrange("(j p) e -> p j e", p=128))
    wv_sb = const.tile([128, ND, HL * DK], F32)
    nc.sync.dma_start(out=wv_sb, in_=wv_d.rearrange("(j p) e -> p j e", p=128))
    wo01_sb = const.tile([128, D], F32)  # head0 rows on 0:64, head1 on 64:128
    nc.sync.dma_start(out=wo01_sb, in_=wo_d[0:2].rearrange("h p d -> (h p) d"))
    wo2_sb = const.tile([DK, D], F32)
    nc.sync.dma_start(out=wo2_sb, in_=wo_d[2])

    # ---- persistent activations ----
    # KA: [K^T_h0 ; K^T_h1], QB: [Q^T_h0 ; Q^T_h1] on partition halves
    KA = persist.tile([128, T], F32, name="KA")
    QB = persist.tile([128, T], F32, name="QB")
    C2 = persist.tile([128, T], F32, name="C2")  # [K^T_h2 ; Q^T_h2]
    D2 = persist.tile([128, T], F32, name="D2")  # [Q^T_h2 ; K^T_h2] (swapped copy)
    Vh = []
    for h in range(HL):
        vt = persist.tile([128, NKT, DK + 1], F32, name=f"V{h}")
        nc.gpsimd.memset(vt[:, :, DK : DK + 1], 1.0)  # ones row -> softmax sums
        Vh.append(vt)
    ot01 = persist.tile([128, 512], F32, name="ot01")  # heads 0/1 out^T per qb
    ot2 = persist.tile([DK, 512], F32, name="ot2")

    qk_dest = [KA, QB, C2]

    # ================= phase A: projections =================
    for tsb in range(NTSB):
        xt_sb = xtpool.tile([128, ND, 512], F32, name="xt_sb")
        for tt in range(4):
            t0 = (tsb * 4 + tt) * 128
            x_sb = xpool.tile([128, D], F32, name="x_sb")
            nc.sync.dma_start(out=x_sb, in_=x_d[t0 : t0 + 128, :])
            ps_t = psA.tile([128, ND * 128], F32, name="ps_t", tag="pa")
            for dj in range(ND):
                nc.tensor.transpose(
                    ps_t[:, dj * 128 : (dj + 1) * 128],
                    x_sb[:, dj * 128 : (dj + 1) * 128],
                    ident,
                )
            nc.vector.tensor_copy(
                xt_sb[:, :, tt * 128 : (tt + 1) * 128],
                ps_t.rearrange("p (j t) -> p j t", j=ND),
            )
        # Q^T / K^T projection: out[e, t] block per e-tile
        for et in range(3):
            ps_q = psA.tile([128, 512], F32, name="ps_q", tag="pa")
            nc.vector.memset(ps_q, 0.0)
            for dj in range(ND):
                e0 = et * 128
                nc.tensor.matmul(
                    ps_q[0:64, :],
                    lhsT=wqk_sb[:, dj, e0 : e0 + 64],
                    rhs=xt_sb[:, dj, :],
                    start=False, stop=(dj == ND - 1), skip_group_check=True,
                )
                nc.tensor.matmul(
                    ps_q[64:128, :],
                    lhsT=wqk_sb[:, dj, e0 + 64 : e0 + 128],
                    rhs=xt_sb[:, dj, :],
                    start=False, stop=(dj == ND - 1), skip_group_check=True,
                )
            nc.vector.tensor_copy(qk_dest[et][:, tsb * 512 : (tsb + 1) * 512], ps_q)
        # D2 = partition-swapped copy of C2 (for self-paired row-tiling of h2)
        blk = slice(tsb * 512, (tsb + 1) * 512)
        nc.sync.dma_start(out=D2[0:64, blk], in_=C2[64:128, blk])
        nc.sync.dma_start(out=D2[64:128, blk], in_=C2[0:64, blk])
        # V natural: stationary x^T tiles, streaming Wv^T
        for tt in range(4):
            ps_v = psA.tile([128, HL * DK], F32, name="ps_v", tag="pa")
            nc.vector.memset(ps_v, 0.0)
            for dj in range(ND):
                tcol = tt * 128
                nc.tensor.matmul(
                    ps_v[0:64, :],
                    lhsT=xt_sb[:, dj, tcol : tcol + 64],
                    rhs=wv_sb[:, dj, :],
                    start=False, stop=(dj == ND - 1), skip_group_check=True,
                )
                nc.tensor.matmul(
                    ps_v[64:128, :],
                    lhsT=xt_sb[:, dj, tcol + 64 : tcol + 128],
                    rhs=wv_sb[:, dj, :],
                    start=False, stop=(dj == ND - 1), skip_group_check=True,
                )
            kt = tsb * 4 + tt
            for h in range(HL):
                nc.vector.tensor_copy(
                    Vh[h][:, kt, 0:DK], ps_v[:, h * DK : (h + 1) * DK]
                )

    # ================= phase B: attention =================
    for qb in range(NQB):
        nk = 4 * (qb + 1)
        qblk = slice(qb * 512, (qb + 1) * 512)
        ot_slices = {}
        # pass 0: heads (0, 1) row-paired; pass 1: head 2 self-paired
        for hpass, heads in enumerate([(0, 1), (2,)]):
            psav = {h: psB.tile([DK + 1, 512], F32, name=f"psav{h}", tag="pb")
                    for h in heads}
            for kp in range(nk // 2):
                kt0, kt1 = 2 * kp, 2 * kp + 1
                ss = {h: psA.tile([128, 1024], F32, name=f"ss{h}", tag="pa")
                      for h in heads}
                for i, kt in enumerate((kt0, kt1)):
                    kblk = slice(kt * 128, (kt + 1) * 128)
                    off = slice(i * 512, (i + 1) * 512)
                    if hpass == 0:
                        nc.tensor.matmul(
                            ss[0][:, off], lhsT=KA[0:64, kblk],
                            rhs=QB[0:64, qblk], start=True, stop=True,
                        )
                        nc.tensor.matmul(
                            ss[1][:, off], lhsT=KA[64:128, kblk],
                            rhs=QB[64:128, qblk], start=True, stop=True,
                        )
                    elif i == 0:
                        nc.tensor.matmul(
                            ss[2][:, off], lhsT=C2[0:64, kblk],
                            rhs=D2[0:64, qblk], start=True, stop=True,
                        )
                    else:
                        nc.tensor.matmul(
                            ss[2][:, off], lhsT=D2[64:128, kblk],
                            rhs=C2[64:128, qblk], start=True, stop=True,
                        )
                for h in heads:
                    pt = ptpool.tile([128, 1024], F32, name="pt")
                    nc.scalar.activation(pt, ss[h], AF.Exp, scale=0.125)
                    for i, kt in enumerate((kt0, kt1)):
                        off = slice(i * 512, (i + 1) * 512)
                        if kt >= 4 * qb:  # diagonal band tile
                            bp = kt - 4 * qb
                            nc.vector.tensor_mul(
                                pt[:, off], pt[:, off], bandmask[bp]
                            )
                        # AV with the ones row appended to V: row 64 of the
                        # accumulator collects the softmax denominators
                        nc.tensor.matmul(
                            psav[h],
                            lhsT=Vh[h][:, kt, :], rhs=pt[:, off],
                            start=(kt == 0), stop=(kt == nk - 1),
                        )
            # normalize: out^T = (even + odd halves) / sums
            for h in heads:
                sums_sb = spool.tile([1, 512], F32, name="sums_sb")
                nc.vector.tensor_copy(sums_sb, psav[h][DK : DK + 1, :])
                chop = spool.tile([128, 4], F32, name="chop")
                nc.sync.dma_start(out=chop, in_=sums_sb)
                recipC = spool.tile([128, 4], F32, name="recipC")
                nc.vector.reciprocal(recipC, chop)
                recipR = spool.tile([1, 512], F32, name="recipR")
                nc.sync.dma_start(out=recipR, in_=recipC)
                recipb = spool.tile([DK, 512], F32, name="recipb")
                if USE_PB:
                    nc.gpsimd.partition_broadcast(recipb, recipR, channels=DK)
                else:
                    ps_b = psC.tile([128, 512], F32, name="ps_b", tag="pc")
                    nc.tensor.matmul(
                        ps_b[0:DK, :], lhsT=ones64, rhs=recipR,
                        start=True, stop=True,
                    )
                    nc.vector.tensor_copy(recipb, ps_b[0:DK, :])
                if h == 0:
                    nc.vector.tensor_mul(ot01[0:DK, :], psav[h][0:DK, :], recipb)
                    ot_slices[0] = ot01[0:DK, :]
                elif h == 1:
                    ot1s = spool.tile([DK, 512], F32, name="ot1s")
                    nc.vector.tensor_mul(ot1s, psav[h][0:DK, :], recipb)
                    nc.sync.dma_start(out=ot01[DK:128, :], in_=ot1s)
                    ot_slices[1] = ot01[DK:128, :]
                else:
                    nc.vector.tensor_mul(ot2, psav[h][0:DK, :], recipb)
                    ot_slices[2] = ot2
        # out-proj: y^T[d, q] — heads 0/1 stacked on partition halves form a
        # single K=128 contraction; then head 2's K=64 accumulates on top.
        # (Mixed ROW positions inside one accumulation group crash the HW,
        # so never pair row-groups within an accumulating chain.)
        for dj in range(ND):
            dblk = slice(dj * 128, (dj + 1) * 128)
            ps_y = psC.tile([128, 512], F32, name="ps_y", tag="pc")
            nc.tensor.matmul(
                ps_y, lhsT=wo01_sb[:, dblk], rhs=ot01,
                start=True, stop=False, skip_group_check=True,
            )
            nc.tensor.matmul(
                ps_y, lhsT=wo2_sb[:, dblk], rhs=ot2,
                start=False, stop=True, skip_group_check=True,
            )
            y_sb = ypool.tile([128, 512], F32, name="y_sb")
            nc.vector.tensor_copy(y_sb, ps_y)
            nc.sync.dma_start(out=y_d[dblk, qblk], in_=y_sb)
    ctx.close()


def build():
    if "nc" in _CACHE:
        return _CACHE["nc"]
    nc = bacc.Bacc(
        "TRN2", target_bir_lowering=False, debug=False, num_devices=NCORES
    )
    with tile.TileContext(nc) as tc:
        _emit(tc)
    nc.compile()
    _CACHE["nc"] = nc
    return nc


def make_in_maps(x, w_qkv, w_out):
    x = np.asarray(x, dtype=np.float32)
    w_qkv = np.asarray(w_qkv, dtype=np.float32)
    w_out = np.asarray(w_out, dtype=np.float32)
    wq = w_qkv[0:D]        # [768, 768], rows = q features
    wk = w_qkv[D : 2 * D]
    wv = w_qkv[2 * D :]
    in_maps = []
    for c in range(NCORES):
        b, g = divmod(c, 4)
        hs = [3 * g + j for j in range(HL)]  # global head ids
        h0, h1, h2 = hs
        cols = []
        for pair in ((wk, h0), (wk, h1), (wq, h0), (wq, h1), (wk, h2), (wq, h2)):
            w, h = pair
            cols.append(w[h * DK : (h + 1) * DK].T)  # [768, 64]
        wqkT = np.ascontiguousarray(np.concatenate(cols, axis=1))  # [768, 384]
        wvT = np.ascontiguousarray(
            np.concatenate([wv[h * DK : (h + 1) * DK].T for h in hs], axis=1)
        )  # [768, 192]
        woT = np.ascontiguousarray(
            np.stack([w_out[:, h * DK : (h + 1) * DK].T for h in hs])
        )  # [3, 64, 768]
        in_maps.append(
            {
                "x": np.ascontiguousarray(x[b]),
                "wqkT": wqkT,
                "wvT": wvT,
                "woT": woT,
            }
        )
    return in_maps


def run(inputs, trace=False):
    """Run on hardware; returns (y [B,T,D] fp32, BassKernelResults)."""
    nc = build()
    in_maps = make_in_maps(inputs["x"], inputs["w_qkv"], inputs["w_out"])
    br = run_bass_kernel_spmd(nc, in_maps, list(range(NCORES)), trace=trace)
    y = np.zeros((B, T, D), dtype=np.float32)
    for c in range(NCORES):
        b = c // 4
        y[b] += np.asarray(br.results[c]["yT"]).T
    return y, br


def kernel(x, w_qkv, w_out):
    y, _ = run({"x": x, "w_qkv": w_qkv, "w_out": w_out})
    return y



# revision 2
# speedup vs baseline: 2.5130x; 2.5130x over previous
"""Multi-head causal self-attention (B=2, T=4096, D=768, H=12) on 8 trn2 cores.

Sharding: core c -> batch b = c//4, heads 3*(c%4) .. 3*(c%4)+2.
qkv_proj column-parallel (each core computes Q/K/V only for its heads),
out_proj row-parallel (each core emits a partial y^T; host sums the 4
partials per batch).

v2: all matmuls in bf16 (1 cyc/row vs fp32's 4), x transposed + converted
to bf16 on the host (kills the PE-transpose phase), 128-wide stationary
projection tiles, psum accumulation via start/stop (no memsets).

Device dataflow:
  x^T bf16 DMA'd per 512-col chunk -> Q^T/K^T via transposed projection
  (W^T stationary, x^T streaming) -> V natural (x^T chunks stationary,
  Wv^T streaming) -> S^T = K Q^T in [k,q] layout, two heads row-paired on
  opposite PE halves -> exp on ScalarE (no max subtraction; scores ~
  N(0,1)) writing bf16 -> causal band masks on DVE -> out^T = V^T P^T
  with a ones column appended to V collecting softmax denominators in
  psum row 64 -> normalize via batched reciprocal + gpsimd partition
  broadcast -> y^T = Wo^T.T out^T with heads 0/1 row-paired.
"""

import sys

sys.path.insert(0, "/opt/trn_rl_repo")

import numpy as np
import ml_dtypes
from contextlib import ExitStack

import concourse.bass as bass
import concourse.bacc as bacc
import concourse.tile as tile
import concourse.mybir as mybir
from concourse.bass_utils import run_bass_kernel_spmd

F32 = mybir.dt.float32
BF16 = mybir.dt.bfloat16
NPBF = np.dtype(ml_dtypes.bfloat16)
AF = mybir.ActivationFunctionType

B = 2
T = 4096
D = 768
H = 12
DK = 64
NCORES = 8
HL = 3  # heads per core
ND = D // 128  # 6 d-tiles
NKT = T // 128  # 32 k-tiles
NQB = T // 512  # 8 q-blocks
NTSB = T // 512  # 8 t-superblocks

_CACHE = {}


def _emit(tc):
    nc = tc.nc
    xT_d = nc.dram_tensor("xT", [D, T], BF16, kind="ExternalInput").ap()
    wqk_d = nc.dram_tensor("wqkT", [D, 384], BF16, kind="ExternalInput").ap()
    wv_d = nc.dram_tensor("wvT", [D, HL * DK], BF16, kind="ExternalInput").ap()
    wo_d = nc.dram_tensor("woT", [HL, DK, D], BF16, kind="ExternalInput").ap()
    y_d = nc.dram_tensor("yT", [D, T], F32, kind="ExternalOutput").ap()

    ctx = ExitStack()
    const = ctx.enter_context(tc.tile_pool(name="const", bufs=1))
    persist = ctx.enter_context(tc.tile_pool(name="persist", bufs=1))
    ptpool = ctx.enter_context(tc.tile_pool(name="pt", bufs=5))
    spool = ctx.enter_context(tc.tile_pool(name="sp", bufs=2))
    ypool = ctx.enter_context(tc.tile_pool(name="yp", bufs=2))
    # PSUM: pa = streaming (S tiles, qkv, V); pb = AV accumulators; pc =
    # out-proj. 2*2 + 3*1 + 1 = 8 banks.
    psA = ctx.enter_context(tc.tile_pool(name="psA", bufs=2, space="PSUM"))
    psB = ctx.enter_context(tc.tile_pool(name="psB", bufs=3, space="PSUM"))
    psC = ctx.enter_context(tc.tile_pool(name="psC", bufs=1, space="PSUM"))

    # ---- constants ----
    # causal band masks for the 4 diagonal-band k-tiles of each q-block:
    # bandmask[bp][k, q] = 0 for q < 128*bp + k, else 1
    bandmask = []
    for bp in range(4):
        m = const.tile([128, 512], BF16, name=f"bandmask{bp}")
        nc.gpsimd.memset(m, 1.0)
        nc.gpsimd.affine_select(
            out=m, in_=m, compare_op=mybir.AluOpType.is_ge, fill=0.0,
            base=-128 * bp, pattern=[[1, 512]], channel_multiplier=-1,
        )
        bandmask.append(m)

    wqk_sb = const.tile([128, ND, 384], BF16)
    nc.sync.dma_start(out=wqk_sb, in_=wqk_d.rearrange("(j p) e -> p j e", p=128))
    wv_sb = const.tile([128, ND, HL * DK], BF16)
    nc.sync.dma_start(out=wv_sb, in_=wv_d.rearrange("(j p) e -> p j e", p=128))
    wo01_sb = const.tile([128, D], BF16)  # head0 rows on 0:64, head1 on 64:128
    nc.sync.dma_start(out=wo01_sb, in_=wo_d[0:2].rearrange("h p d -> (h p) d"))
    wo2_sb = const.tile([DK, D], BF16)
    nc.sync.dma_start(out=wo2_sb, in_=wo_d[2])

    # ---- persistent activations ----
    xT_sb = persist.tile([128, ND, T], BF16, name="xT")
    # KA: [K^T_h0 ; K^T_h1], QB: [Q^T_h0 ; Q^T_h1] on partition halves
    KA = persist.tile([128, T], BF16, name="KA")
    QB = persist.tile([128, T], BF16, name="QB")
    C2 = persist.tile([128, T], BF16, name="C2")  # [K^T_h2 ; Q^T_h2]
    D2 = persist.tile([128, T], BF16, name="D2")  # [Q^T_h2 ; K^T_h2] (swapped)
    V = persist.tile([128, HL, NKT, DK + 1], BF16, name="V")
    nc.gpsimd.memset(V[:, :, :, DK : DK + 1], 1.0)  # ones col -> softmax sums
    ot01 = persist.tile([128, 512], BF16, name="ot01")  # heads 0/1 out^T per qb
    ot2 = persist.tile([DK, 512], BF16, name="ot2")

    qk_dest = [KA, QB, C2]
    xT_dr = xT_d.rearrange("(j p) t -> p j t", p=128)

    # ================= phase A: projections =================
    for tsb in range(NTSB):
        blk = slice(tsb * 512, (tsb + 1) * 512)
        nc.sync.dma_start(out=xT_sb[:, :, blk], in_=xT_dr[:, :, blk])
        # Q^T / K^T projection: out[e, t] block per e-tile
        for et in range(3):
            ps_q = psA.tile([128, 512], F32, name="ps_q", tag="pa")
            e0 = et * 128
            for dj in range(ND):
                nc.tensor.matmul(
                    ps_q,
                    lhsT=wqk_sb[:, dj, e0 : e0 + 128],
                    rhs=xT_sb[:, dj, blk],
                    start=(dj == 0), stop=(dj == ND - 1),
                )
            nc.vector.tensor_copy(qk_dest[et][:, blk], ps_q)
        # D2 = partition-swapped copy of C2 (for self-paired row-tiling of h2)
        nc.sync.dma_start(out=D2[0:64, blk], in_=C2[64:128, blk])
        nc.sync.dma_start(out=D2[64:128, blk], in_=C2[0:64, blk])
        # V natural: stationary x^T tiles, streaming Wv^T
        for tt in range(4):
            t0 = (tsb * 4 + tt) * 128
            ps_v = psA.tile([128, HL * DK], F32, name="ps_v", tag="pa")
            for dj in range(ND):
                nc.tensor.matmul(
                    ps_v,
                    lhsT=xT_sb[:, dj, t0 : t0 + 128],
                    rhs=wv_sb[:, dj, :],
                    start=(dj == 0), stop=(dj == ND - 1),
                )
            kt = tsb * 4 + tt
            nc.vector.tensor_copy(
                V[:, :, kt, 0:DK], ps_v.rearrange("p (h c) -> p h c", h=HL)
            )

    # ================= phase B: attention =================
    for qb in range(NQB):
        nk = 4 * (qb + 1)
        qblk = slice(qb * 512, (qb + 1) * 512)
        # pass 0: heads (0, 1) row-paired; pass 1: head 2 self-paired
        for hpass, heads in enumerate([(0, 1), (2,)]):
            psav = {h: psB.tile([DK + 1, 512], F32, name=f"psav{h}", tag="pb")
                    for h in heads}
            for kp in range(nk // 2):
                kt0, kt1 = 2 * kp, 2 * kp + 1
                ss = {h: psA.tile([128, 1024], F32, name=f"ss{h}", tag="pa")
                      for h in heads}
                for i, kt in enumerate((kt0, kt1)):
                    kblk = slice(kt * 128, (kt + 1) * 128)
                    off = slice(i * 512, (i + 1) * 512)
                    if hpass == 0:
                        nc.tensor.matmul(
                            ss[0][:, off], lhsT=KA[0:64, kblk],
                            rhs=QB[0:64, qblk], start=True, stop=True,
                        )
                        nc.tensor.matmul(
                            ss[1][:, off], lhsT=KA[64:128, kblk],
                            rhs=QB[64:128, qblk], start=True, stop=True,
                        )
                    elif i == 0:
                        nc.tensor.matmul(
                            ss[2][:, off], lhsT=C2[0:64, kblk],
                            rhs=D2[0:64, qblk], start=True, stop=True,
                        )
                    else:
                        nc.tensor.matmul(
                            ss[2][:, off], lhsT=D2[64:128, kblk],
                            rhs=C2[64:128, qblk], start=True, stop=True,
                        )
                for h in heads:
                    pt = ptpool.tile([128, 1024], BF16, name="pt")
                    nc.scalar.activation(pt, ss[h], AF.Exp, scale=0.125)
                    for i, kt in enumerate((kt0, kt1)):
                        off = slice(i * 512, (i + 1) * 512)
                        if kt >= 4 * qb:  # diagonal band tile
                            bp = kt - 4 * qb
                            nc.vector.tensor_mul(
                                pt[:, off], pt[:, off], bandmask[bp]
                            )
                        # AV with the ones column appended to V: row 64 of
                        # the accumulator collects the softmax denominators
                        nc.tensor.matmul(
                            psav[h],
                            lhsT=V[:, h, kt, :], rhs=pt[:, off],
                            start=(kt == 0), stop=(kt == nk - 1),
                        )
            # normalize: out^T = psav rows 0:64 / sums (row 64)
            for h in heads:
                sums_sb = spool.tile([1, 512], F32, name="sums_sb")
                nc.vector.tensor_copy(sums_sb, psav[h][DK : DK + 1, :])
                chop = spool.tile([128, 4], F32, name="chop")
                nc.sync.dma_start(out=chop, in_=sums_sb)
                recipC = spool.tile([128, 4], F32, name="recipC")
                nc.vector.reciprocal(recipC, chop)
                recipR = spool.tile([1, 512], F32, name="recipR")
                nc.sync.dma_start(out=recipR, in_=recipC)
                recipb = spool.tile([DK, 512], F32, name="recipb")
                nc.gpsimd.partition_broadcast(recipb, recipR, channels=DK)
                if h == 0:
                    nc.vector.tensor_mul(ot01[0:DK, :], psav[h][0:DK, :], recipb)
                elif h == 1:
                    ot1s = spool.tile([DK, 512], BF16, name="ot1s")
                    nc.vector.tensor_mul(ot1s, psav[h][0:DK, :], recipb)
                    nc.sync.dma_start(out=ot01[DK:128, :], in_=ot1s)
                else:
                    nc.vector.tensor_mul(ot2, psav[h][0:DK, :], recipb)
        # out-proj: y^T[d, q] — heads 0/1 stacked on partition halves form a
        # single K=128 contraction; then head 2's K=64 accumulates on top.
        # (Mixed ROW positions inside one accumulation group crash the HW,
        # so never pair row-groups within an accumulating chain.)
        for dj in range(ND):
            dblk = slice(dj * 128, (dj + 1) * 128)
            ps_y = psC.tile([128, 512], F32, name="ps_y", tag="pc")
            nc.tensor.matmul(
                ps_y, lhsT=wo01_sb[:, dblk], rhs=ot01,
                start=True, stop=False, skip_group_check=True,
            )
            nc.tensor.matmul(
                ps_y, lhsT=wo2_sb[:, dblk], rhs=ot2,
                start=False, stop=True, skip_group_check=True,
            )
            y_sb = ypool.tile([128, 512], F32, name="y_sb")
            nc.vector.tensor_copy(y_sb, ps_y)
            nc.sync.dma_start(out=y_d[dblk, qblk], in_=y_sb)
    ctx.close()


def build():
    if "nc" in _CACHE:
        return _CACHE["nc"]
    nc = bacc.Bacc(
        "TRN2", target_bir_lowering=False, debug=False, num_devices=NCORES
    )
    with tile.TileContext(nc) as tc:
        _emit(tc)
    nc.compile()
    _CACHE["nc"] = nc
    return nc


def make_in_maps(x, w_qkv, w_out):
    x = np.asarray(x, dtype=np.float32)
    w_qkv = np.asarray(w_qkv, dtype=np.float32)
    w_out = np.asarray(w_out, dtype=np.float32)
    wq = w_qkv[0:D]        # [768, 768], rows = q features
    wk = w_qkv[D : 2 * D]
    wv = w_qkv[2 * D :]
    xT = [np.ascontiguousarray(x[b].T).astype(NPBF) for b in range(B)]
    in_maps = []
    for c in range(NCORES):
        b, g = divmod(c, 4)
        hs = [3 * g + j for j in range(HL)]  # global head ids
        h0, h1, h2 = hs
        cols = []
        for pair in ((wk, h0), (wk, h1), (wq, h0), (wq, h1), (wk, h2), (wq, h2)):
            w, h = pair
            cols.append(w[h * DK : (h + 1) * DK].T)  # [768, 64]
        wqkT = np.ascontiguousarray(np.concatenate(cols, axis=1)).astype(NPBF)
        wvT = np.ascontiguousarray(
            np.concatenate([wv[h * DK : (h + 1) * DK].T for h in hs], axis=1)
        ).astype(NPBF)  # [768, 192]
        woT = np.ascontiguousarray(
            np.stack([w_out[:, h * DK : (h + 1) * DK].T for h in hs])
        ).astype(NPBF)  # [3, 64, 768]
        in_maps.append(
            {
                "xT": xT[b],
                "wqkT": wqkT,
                "wvT": wvT,
                "woT": woT,
            }
        )
    return in_maps


def run(inputs, trace=False):
    """Run on hardware; returns (y [B,T,D] fp32, BassKernelResults)."""
    nc = build()
    in_maps = make_in_maps(inputs["x"], inputs["w_qkv"], inputs["w_out"])
    br = run_bass_kernel_spmd(nc, in_maps, list(range(NCORES)), trace=trace)
    y = np.zeros((B, T, D), dtype=np.float32)
    for c in range(NCORES):
        b = c // 4
        y[b] += np.asarray(br.results[c]["yT"], dtype=np.float32).T
    return y, br


def kernel(x, w_qkv, w_out):
    y, _ = run({"x": x, "w_qkv": w_qkv, "w_out": w_out})
    return y


# revision 4
# speedup vs baseline: 3.4079x; 1.3561x over previous
"""Multi-head causal self-attention (B=2, T=4096, D=768, H=12) on 8 trn2 cores.

Sharding: core c -> batch b = c//4, heads 3*(c%4) .. 3*(c%4)+2.
qkv_proj column-parallel (each core computes Q/K/V only for its heads),
out_proj row-parallel (each core emits a partial y^T; host sums the 4
partials per batch).

v3: bf16 matmuls + host-side x^T; software-pipelined schedule: projection
and out-proj matmuls are injected as PE "filler" work between attention
k-pair steps so the tensor engine never idles (keeps the DVFS p-state at
max); causal band tiles trim their invalid columns from the scores/exp/AV
work (single 128x128 triangle mask replaces the wide band masks); AV psum
accumulators are copied to SBUF immediately so their banks recycle fast.

Device dataflow:
  x^T bf16 DMA'd per 512-col chunk -> Q^T/K^T via transposed projection
  (W^T stationary, x^T streaming) -> V natural (x^T chunks stationary,
  Wv^T streaming) -> S^T = K Q^T in [k,q] layout, two heads row-paired on
  opposite PE halves -> exp on ScalarE writing bf16 -> triangle masks on
  DVE for diagonal tiles -> out^T = V^T P^T with a ones column appended
  to V collecting softmax denominators in psum row 64 -> normalize via
  batched reciprocal + gpsimd partition broadcast (double-buffered by qb
  parity) -> y^T = Wo^T.T out^T, deferred one q-block as filler work.
"""

import sys

sys.path.insert(0, "/opt/trn_rl_repo")

import numpy as np
import ml_dtypes
from collections import deque
from contextlib import ExitStack

import concourse.bass as bass
import concourse.bacc as bacc
import concourse.tile as tile
import concourse.mybir as mybir
from concourse.bass_utils import run_bass_kernel_spmd

F32 = mybir.dt.float32
BF16 = mybir.dt.bfloat16
NPBF = np.dtype(ml_dtypes.bfloat16)
AF = mybir.ActivationFunctionType

B = 2
T = 4096
D = 768
H = 12
DK = 64
NCORES = 8
HL = 3  # heads per core
ND = D // 128  # 6 d-tiles
NKT = T // 128  # 32 k-tiles
NQB = T // 512  # 8 q-blocks
NTSB = T // 512  # 8 t-superblocks

_CACHE = {}


def _emit(tc):
    nc = tc.nc
    xT_d = nc.dram_tensor("xT", [D, T], BF16, kind="ExternalInput").ap()
    wqk_d = nc.dram_tensor("wqkT", [D, 384], BF16, kind="ExternalInput").ap()
    wv_d = nc.dram_tensor("wvT", [D, HL * DK], BF16, kind="ExternalInput").ap()
    wo_d = nc.dram_tensor("woT", [HL, DK, D], BF16, kind="ExternalInput").ap()
    y_d = nc.dram_tensor("yT", [D, T], F32, kind="ExternalOutput").ap()

    ctx = ExitStack()
    const = ctx.enter_context(tc.tile_pool(name="const", bufs=1))
    persist = ctx.enter_context(tc.tile_pool(name="persist", bufs=1))
    ptpool = ctx.enter_context(tc.tile_pool(name="pt", bufs=6))
    spool = ctx.enter_context(tc.tile_pool(name="sp", bufs=2))
    ypool = ctx.enter_context(tc.tile_pool(name="yp", bufs=2))
    # PSUM (8 banks): psA 2x[128,1024]f32 = 4 for score tiles; psB 2x1 for
    # AV accumulators; psC 2x1 shared by projection / out-proj fillers.
    psA = ctx.enter_context(tc.tile_pool(name="psA", bufs=2, space="PSUM"))
    psB = ctx.enter_context(tc.tile_pool(name="psB", bufs=2, space="PSUM"))
    psC = ctx.enter_context(tc.tile_pool(name="psC", bufs=2, space="PSUM"))

    # ---- constants ----
    # triangle mask for the first 128 valid columns of each diagonal band
    # tile: tri[k, j] = 1 for j >= k else 0
    tri = const.tile([128, 128], BF16, name="tri")
    nc.gpsimd.memset(tri, 1.0)
    nc.gpsimd.affine_select(
        out=tri, in_=tri, compare_op=mybir.AluOpType.is_ge, fill=0.0,
        base=0, pattern=[[1, 128]], channel_multiplier=-1,
    )

    wqk_sb = const.tile([128, ND, 384], BF16)
    nc.sync.dma_start(out=wqk_sb, in_=wqk_d.rearrange("(j p) e -> p j e", p=128))
    wv_sb = const.tile([128, ND, HL * DK], BF16)
    nc.sync.dma_start(out=wv_sb, in_=wv_d.rearrange("(j p) e -> p j e", p=128))
    wo01_sb = const.tile([128, D], BF16)  # head0 rows on 0:64, head1 on 64:128
    nc.sync.dma_start(out=wo01_sb, in_=wo_d[0:2].rearrange("h p d -> (h p) d"))
    wo2_sb = const.tile([DK, D], BF16)
    nc.sync.dma_start(out=wo2_sb, in_=wo_d[2])

    # ---- persistent activations ----
    xT_sb = persist.tile([128, ND, T], BF16, name="xT")
    # KA: [K^T_h0 ; K^T_h1], QB: [Q^T_h0 ; Q^T_h1] on partition halves
    KA = persist.tile([128, T], BF16, name="KA")
    QB = persist.tile([128, T], BF16, name="QB")
    C2 = persist.tile([128, T], BF16, name="C2")  # [K^T_h2 ; Q^T_h2]
    D2 = persist.tile([128, T], BF16, name="D2")  # [Q^T_h2 ; K^T_h2] (swapped)
    V = persist.tile([128, HL, NKT, DK + 1], BF16, name="V")
    nc.gpsimd.memset(V[:, :, :, DK : DK + 1], 1.0)  # ones col -> softmax sums
    # out^T staging, double-buffered by q-block parity (out-proj is deferred
    # into the next q-block's filler slots)
    ot01 = [persist.tile([128, 512], BF16, name=f"ot01_{p}") for p in range(2)]
    ot2 = [persist.tile([DK, 512], BF16, name=f"ot2_{p}") for p in range(2)]

    qk_dest = [KA, QB, C2]
    xT_dr = xT_d.rearrange("(j p) t -> p j t", p=128)

    # ---- filler items: projection work for one t-superblock ----
    def proj_fillers(tsb):
        blk = slice(tsb * 512, (tsb + 1) * 512)

        def dma_item():
            nc.sync.dma_start(out=xT_sb[:, :, blk], in_=xT_dr[:, :, blk])

        def qk_item(et):
            ps_q = psC.tile([128, 512], F32, name="ps_q", tag="pc")
            e0 = et * 128
            for dj in range(ND):
                nc.tensor.matmul(
                    ps_q,
                    lhsT=wqk_sb[:, dj, e0 : e0 + 128],
                    rhs=xT_sb[:, dj, blk],
                    start=(dj == 0), stop=(dj == ND - 1),
                )
            nc.vector.tensor_copy(qk_dest[et][:, blk], ps_q)
            if et == 2:
                # D2 = partition-swapped copy of C2 (h2 self-pairing)
                nc.sync.dma_start(out=D2[0:64, blk], in_=C2[64:128, blk])
                nc.sync.dma_start(out=D2[64:128, blk], in_=C2[0:64, blk])

        def v_item(tt):
            t0 = (tsb * 4 + tt) * 128
            ps_v = psC.tile([128, HL * DK], F32, name="ps_v", tag="pc")
            for dj in range(ND):
                nc.tensor.matmul(
                    ps_v,
                    lhsT=xT_sb[:, dj, t0 : t0 + 128],
                    rhs=wv_sb[:, dj, :],
                    start=(dj == 0), stop=(dj == ND - 1),
                )
            kt = tsb * 4 + tt
            nc.vector.tensor_copy(
                V[:, :, kt, 0:DK], ps_v.rearrange("p (h c) -> p h c", h=HL)
            )

        items = [dma_item]
        items += [lambda et=et: qk_item(et) for et in range(3)]
        items += [lambda tt=tt: v_item(tt) for tt in range(4)]
        return items

    # ---- filler items: out-projection of one q-block ----
    def outproj_fillers(qb):
        qblk = slice(qb * 512, (qb + 1) * 512)
        o01, o2 = ot01[qb % 2], ot2[qb % 2]

        def y_item(dj):
            dblk = slice(dj * 128, (dj + 1) * 128)
            ps_y = psC.tile([128, 512], F32, name="ps_y", tag="pc")
            nc.tensor.matmul(
                ps_y, lhsT=wo01_sb[:, dblk], rhs=o01,
                start=True, stop=False, skip_group_check=True,
            )
            nc.tensor.matmul(
                ps_y, lhsT=wo2_sb[:, dblk], rhs=o2,
                start=False, stop=True, skip_group_check=True,
            )
            y_sb = ypool.tile([128, 512], F32, name="y_sb")
            nc.vector.tensor_copy(y_sb, ps_y)
            nc.sync.dma_start(out=y_d[dblk, qblk], in_=y_sb)

        return [lambda dj=dj: y_item(dj) for dj in range(ND)]

    # fq_proj: hard deadline (drained before the attention block that reads
    # it); fq_out: deferred out-proj, drains opportunistically.
    fq_proj = deque()
    fq_out = deque()

    def emit_fillers(n):
        for _ in range(n):
            if fq_proj:
                fq_proj.popleft()()
            elif fq_out:
                fq_out.popleft()()
            else:
                return

    # warm-up: project t-superblock 0 before attention starts
    for it in proj_fillers(0):
        it()
    fq_proj.extend(proj_fillers(1))

    # ================= attention, pipelined =================
    for qb in range(NQB):
        nk = 4 * (qb + 1)
        o01, o2 = ot01[qb % 2], ot2[qb % 2]
        # k-pair list; the two diagonal-band pairs are reordered (hi, lo)
        # so the exp range stays a single contiguous span per pt tile
        pairs = [(2 * kp, 2 * kp + 1) for kp in range(nk // 2)]
        pairs[-2] = (nk - 3, nk - 4)
        pairs[-1] = (nk - 1, nk - 2)
        for hpass, heads in enumerate([(0, 1), (2,)]):
            psav = {h: psB.tile([DK + 1, 512], F32, name=f"psav{h}", tag="pb")
                    for h in heads}
            for kt_a, kt_b in pairs:
                ss = {h: psA.tile([128, 1024], F32, name=f"ss{h}", tag="pa")
                      for h in heads}
                lo_a = max(0, kt_a - 4 * qb) * 128  # valid col start (band)
                lo_b = max(0, kt_b - 4 * qb) * 128
                for i, (kt, lo) in enumerate(((kt_a, lo_a), (kt_b, lo_b))):
                    kblk = slice(kt * 128, (kt + 1) * 128)
                    dst = slice(i * 512 + lo, i * 512 + 512)
                    qsub = slice(qb * 512 + lo, (qb + 1) * 512)
                    if hpass == 0:
                        nc.tensor.matmul(
                            ss[0][:, dst], lhsT=KA[0:64, kblk],
                            rhs=QB[0:64, qsub], start=True, stop=True,
                        )
                        nc.tensor.matmul(
                            ss[1][:, dst], lhsT=KA[64:128, kblk],
                            rhs=QB[64:128, qsub], start=True, stop=True,
                        )
                    elif i == 0:
                        nc.tensor.matmul(
                            ss[2][:, dst], lhsT=C2[0:64, kblk],
                            rhs=D2[0:64, qsub], start=True, stop=True,
                        )
                    else:
                        nc.tensor.matmul(
                            ss[2][:, dst], lhsT=D2[64:128, kblk],
                            rhs=C2[64:128, qsub], start=True, stop=True,
                        )
                emit_fillers(1)
                for h in heads:
                    pt = ptpool.tile([128, 1024], BF16, name="pt")
                    # one contiguous exp span [lo_a:1024]; for a reordered
                    # band pair (hi, lo) any gap columns hold junk that the
                    # AV rhs slices below never touch
                    nc.scalar.activation(
                        pt[:, lo_a:1024], ss[h][:, lo_a:1024], AF.Exp,
                        scale=0.125,
                    )
                    # triangle mask on the first 128 valid cols of band tiles
                    for i, (kt, lo) in enumerate(((kt_a, lo_a), (kt_b, lo_b))):
                        if kt >= 4 * qb:
                            c0 = i * 512 + lo
                            nc.vector.tensor_mul(
                                pt[:, c0 : c0 + 128], pt[:, c0 : c0 + 128], tri
                            )
                    # AV ascending kt within the pair (kt==0 carries the
                    # full-width start=True that initializes the bank)
                    for i, kt, lo in sorted(
                        ((0, kt_a, lo_a), (1, kt_b, lo_b)), key=lambda e: e[1]
                    ):
                        nc.tensor.matmul(
                            psav[h][:, lo:512],
                            lhsT=V[:, h, kt, :],
                            rhs=pt[:, i * 512 + lo : i * 512 + 512],
                            start=(kt == 0), stop=(kt == nk - 1),
                            skip_group_check=True,
                        )
                if len(fq_proj) + len(fq_out) > 10:
                    emit_fillers(2)
            # normalize: out^T = psav rows 0:64 / sums (row 64); copy psav
            # to SBUF right away so the psum bank recycles quickly
            for h in heads:
                av_sb = spool.tile([DK, 512], F32, name="av_sb", tag="av")
                nc.vector.tensor_copy(av_sb, psav[h][0:DK, :])
                sums_sb = spool.tile([1, 512], F32, name="sums_sb", tag="sm")
                nc.vector.tensor_copy(sums_sb, psav[h][DK : DK + 1, :])
                chop = spool.tile([128, 4], F32, name="chop", tag="ch")
                nc.sync.dma_start(out=chop, in_=sums_sb)
                recipC = spool.tile([128, 4], F32, name="recipC", tag="rc")
                nc.vector.reciprocal(recipC, chop)
                recipR = spool.tile([1, 512], F32, name="recipR", tag="rr")
                nc.sync.dma_start(out=recipR, in_=recipC)
                recipb = spool.tile([DK, 512], F32, name="recipb", tag="rb")
                nc.gpsimd.partition_broadcast(recipb, recipR, channels=DK)
                if h == 0:
                    nc.vector.tensor_mul(o01[0:DK, :], av_sb, recipb)
                elif h == 1:
                    ot1s = spool.tile([DK, 512], BF16, name="ot1s", tag="o1")
                    nc.vector.tensor_mul(ot1s, av_sb, recipb)
                    nc.sync.dma_start(out=o01[DK:128, :], in_=ot1s)
                else:
                    nc.vector.tensor_mul(o2, av_sb, recipb)
        # hard deadline: projections for the next q-block must be fully
        # emitted before its attention reads KA/QB/C2/D2/V
        emit_n = len(fq_proj)
        for _ in range(emit_n):
            fq_proj.popleft()()
        # defer this q-block's out-projection into upcoming filler slots
        fq_out.extend(outproj_fillers(qb))
        if qb + 2 < NTSB:
            fq_proj.extend(proj_fillers(qb + 2))
    while fq_out:
        fq_out.popleft()()
    ctx.close()


def build():
    if "nc" in _CACHE:
        return _CACHE["nc"]
    nc = bacc.Bacc(
        "TRN2", target_bir_lowering=False, debug=False, num_devices=NCORES
    )
    with tile.TileContext(nc) as tc:
        _emit(tc)
    nc.compile()
    _CACHE["nc"] = nc
    return nc


def make_in_maps(x, w_qkv, w_out):
    x = np.asarray(x, dtype=np.float32)
    w_qkv = np.asarray(w_qkv, dtype=np.float32)
    w_out = np.asarray(w_out, dtype=np.float32)
    wq = w_qkv[0:D]        # [768, 768], rows = q features
    wk = w_qkv[D : 2 * D]
    wv = w_qkv[2 * D :]
    xT = [np.ascontiguousarray(x[b].T).astype(NPBF) for b in range(B)]
    in_maps = []
    for c in range(NCORES):
        b, g = divmod(c, 4)
        hs = [3 * g + j for j in range(HL)]  # global head ids
        h0, h1, h2 = hs
        cols = []
        for pair in ((wk, h0), (wk, h1), (wq, h0), (wq, h1), (wk, h2), (wq, h2)):
            w, h = pair
            cols.append(w[h * DK : (h + 1) * DK].T)  # [768, 64]
        wqkT = np.ascontiguousarray(np.concatenate(cols, axis=1)).astype(NPBF)
        wvT = np.ascontiguousarray(
            np.concatenate([wv[h * DK : (h + 1) * DK].T for h in hs], axis=1)
        ).astype(NPBF)  # [768, 192]
        woT = np.ascontiguousarray(
            np.stack([w_out[:, h * DK : (h + 1) * DK].T for h in hs])
        ).astype(NPBF)  # [3, 64, 768]
        in_maps.append(
            {
                "xT": xT[b],
                "wqkT": wqkT,
                "wvT": wvT,
                "woT": woT,
            }
        )
    return in_maps


def run(inputs, trace=False):
    """Run on hardware; returns (y [B,T,D] fp32, BassKernelResults)."""
    nc = build()
    in_maps = make_in_maps(inputs["x"], inputs["w_qkv"], inputs["w_out"])
    br = run_bass_kernel_spmd(nc, in_maps, list(range(NCORES)), trace=trace)
    y = np.zeros((B, T, D), dtype=np.float32)
    for c in range(NCORES):
        b = c // 4
        y[b] += np.asarray(br.results[c]["yT"], dtype=np.float32).T
    return y, br


def kernel(x, w_qkv, w_out):
    y, _ = run({"x": x, "w_qkv": w_qkv, "w_out": w_out})
    return y
